# revision 1
# baseline (speedup 1.0000x reference)
# Trainium2 Bass kernel for a single-head attention block:
#   qkv = x @ w_attn + b_attn ; q,k,v = split(qkv)
#   out = softmax(q @ k.T / sqrt(H)) @ v @ w_proj + b_proj
# Shapes: x [4, 2048, 1024], w_attn [1024, 3072], w_proj [1024, 1024], f32.
#
# Weight folding (host, input-independent):
#   M    = w_q @ w_k^T          -> scores = x @ M @ x^T (+ column bias)
#   u    = w_k @ b_q            -> the only softmax-surviving bias term
#   w_vt = w_v @ w_proj         -> out = attn @ (x @ w_vt) + beff
#   beff = b_v @ w_proj + b_proj
# (row-dependent bias terms cancel inside softmax). This removes the K
# projection and the output projection: 12.9 GFLOP/core instead of 19.45.
#
# Sharding: 8 cores = 4 batches x 2 sequence-halves. Each core projects
# qM for its query half and vt for its own half; core pairs exchange vt
# halves with an intra-pair AllGather (hidden under the qM projection).
#
# Precision: projections run in bf16; the two attention matmuls run in
# fp8e4m3 with DoubleRow (2 k-tiles per pass). st is stored centered
# (st-1) in fp8 to halve its quantization error; the exact rank-1
# correction ones x colsum(vt8) is added back in the epilogue. Softmax
# sums ride the P4 matmuls as an extra N=1 column; normalization and
# bias stay fp32.
import numpy as np
import ml_dtypes

import concourse.bass as bass
import concourse.mybir as mybir
import concourse.tile as tile
from concourse.bass import ts, ds
from concourse.bass_utils import run_bass_kernel_spmd

P = 128
H = 1024
S = 2048
SQ = 1024  # sequence rows per core (half)
HT = H // P  # 8 h-tiles
ST = S // P  # 16 s-tiles
F32 = mybir.dt.float32
BF16 = mybir.dt.bfloat16
FP8 = mybir.dt.float8e4
AF = mybir.ActivationFunctionType
DR = mybir.MatmulPerfMode.DoubleRow
SCALE = 1.0 / 32.0  # 1/sqrt(H)
GROUPS = [[0, 1], [2, 3], [4, 5], [6, 7]]

P3_FP8 = True  # scores matmul in fp8 DoubleRow
P4_FP8 = True  # attn@vt matmul in fp8 DoubleRow (with st centering)


def _split_excess_waits(nc, limit=1):
    """walrus codegen allows very few sync-wait commands per instruction
    (1 is safe for generic ops, 0 for collectives). Move overflow waits
    onto injected same-engine NoOps just before the offending instruction
    (engines are in-order, so this is equivalent)."""
    n_split = 0
    for f in nc.m.functions:
        for blk in f.blocks:
            il = blk.instructions
            def _limit(inst):
                return 0 if type(inst).__name__ == "InstCollectiveCompute" else limit
            if not any(
                i.sync_info and i.sync_info.on_wait
                and len(i.sync_info.on_wait) > _limit(i)
                for i in il
            ):
                continue
            newl = []
            for inst in il:
                si = inst.sync_info
                lim = _limit(inst)
                if si is not None and si.on_wait and len(si.on_wait) > lim:
                    waits = list(si.on_wait)
                    while len(waits) > lim:
                        take = max(1, limit)
                        chunk, waits = waits[:take], waits[take:]
                        nop = mybir.InstNoOp(
                            name=f"{inst.name}-wsplit{n_split}", ins=[], outs=[]
                        )
                        nop.engine = inst.engine
                        nop.sync_info = mybir.SyncInfo(on_wait=chunk, on_update=[])
                        newl.append(nop)
                        n_split += 1
                    inst.sync_info = mybir.SyncInfo(
                        on_wait=waits, on_update=list(si.on_update)
                    )
                newl.append(inst)
            il[:] = newl
            assert len(blk.instructions) == len(newl)
    return n_split


def _build_nc():
    nc = bass.Bass("TRN2", target_bir_lowering=False, debug=False, num_devices=8)

    KD8 = FP8 if P3_FP8 else BF16  # dtype of the scores operands (xT, qMT)
    AV8 = FP8 if P4_FP8 else BF16  # dtype of st / vt

    xTq_d = nc.dram_tensor("xTq", [H, SQ], BF16, kind="ExternalInput")
    xTf_d = nc.dram_tensor("xTf", [H, S], KD8, kind="ExternalInput")
    M_d = nc.dram_tensor("Mw", [H, H], BF16, kind="ExternalInput")
    wvt_d = nc.dram_tensor("wvt", [H, H], BF16, kind="ExternalInput")
    u_d = nc.dram_tensor("u_cols", [P, HT], F32, kind="ExternalInput")
    beff_d = nc.dram_tensor("beff_bcast", [P, H], F32, kind="ExternalInput")
    out_d = nc.dram_tensor("out", [SQ, H], F32, kind="ExternalOutput")

    xTq_v = xTq_d.ap().rearrange("(j p) s -> p j s", p=P)
    xTf_v = xTf_d.ap().rearrange("(j p) s -> p j s", p=P)
    M_v = M_d.ap().rearrange("(j p) c -> p j c", p=P)
    wvt_v = wvt_d.ap().rearrange("(j p) c -> p j c", p=P)

    from contextlib import ExitStack

    with tile.TileContext(nc) as tc, ExitStack() as top:
        misc = top.enter_context(tc.tile_pool(name="misc", bufs=1))
        dpool = top.enter_context(tc.tile_pool(name="dram", bufs=1, space="DRAM"))

        # staging for the pair AllGather of the vt half (by s-tile)
        stage_d = dpool.tile([HT, P, H], AV8, name="stage_d")
        gath_d = dpool.tile([2, HT, P, H], AV8, name="gath_d")

        u_sb = misc.tile([P, HT], F32, name="u_sb")
        nc.sync.dma_start(u_sb[:, :], u_d.ap())
        beff_sb = misc.tile([P, H], F32, name="beff_sb")
        nc.sync.dma_start(beff_sb[:, :], beff_d.ap())

        # fp8 ones for the sums column (middle-dim step 16 to satisfy DR APs)
        ones3 = misc.tile([P, 2, 16], AV8, name="ones3")
        nc.vector.memset(ones3[:, :, :], 1.0)
        # all-ones stationary for the colsum broadcast matmul
        onesW = misc.tile([P, 2, P], AV8, name="onesW")
        nc.vector.memset(onesW[:, :, :], 1.0)

        # persistent operand buffers
        kq_es = ExitStack()
        kq = kq_es.enter_context(tc.tile_pool(name="kq", bufs=1))
        xTf_sb = kq.tile([P, HT, S], KD8, name="xTf_sb")
        qMT_sb = kq.tile([P, HT, SQ], KD8, name="qMT_sb")
        v_es = ExitStack()
        vp = v_es.enter_context(tc.tile_pool(name="vp", bufs=1, side="right"))
        vt_sb = vp.tile([P, ST, H], AV8, name="vt_sb")

        x_es = ExitStack()
        xTp = x_es.enter_context(tc.tile_pool(name="xTp", bufs=1))
        w_es = ExitStack()
        wpool = w_es.enter_context(tc.tile_pool(name="wpool", bufs=2))

        xTq_sb = xTp.tile([P, HT, SQ], BF16, name="xTq_sb")
        w_v = wpool.tile([P, HT, H], BF16, tag="w", name="w_v")
        # critical chunks first (vt inputs), then the qM inputs
        for j in range(HT):
            nc.sync.dma_start(w_v[:, j, :], wvt_v[:, j, :])
            nc.sync.dma_start(xTq_sb[:, j, :], xTq_v[:, j, :])
        for j in range(HT):
            nc.sync.dma_start(xTf_sb[:, j, :], xTf_v[:, j, :])

        warm_sb = misc.tile([P, 512], BF16, name="warm_sb")
        nc.vector.memset(warm_sb[:, :], 1.0)

        vt_own = xTp.tile([P, HT, H], AV8, name="vt_own")

        with tc.tile_pool(name="p1ps", bufs=4, space="PSUM") as p1ps:
            # PE warm-up on const data while the first DMAs land: keeps the
            # HAM activity window full so real matmuls start at 2.4 GHz.
            for wi in range(4):
                wps = p1ps.tile([P, 512], F32, tag="ps", name=f"warm_ps{wi}")
                for r in range(6):
                    nc.tensor.matmul(
                        wps[:, :],
                        warm_sb[:, 0:P],
                        warm_sb[:, :],
                        start=(r == 0),
                        stop=(r == 5),
                    )
            # --- vt own half: lhsT = xTq tile (stationary), rhs = w_vt ---
            for si in range(HT):
                ps = p1ps.tile([P, H], F32, tag="psv", name=f"psv_{si}", bufs=2)
                for j in range(HT):
                    for hc in range(2):
                        nc.tensor.matmul(
                            ps[:, ds(hc * 512, 512)],
                            xTq_sb[:, j, ts(si, P)],
                            w_v[:, j, ds(hc * 512, 512)],
                            start=(j == 0),
                            stop=(j == HT - 1),
                        )
                nc.scalar.activation(vt_own[:, si, :], ps[:, :], AF.Copy)
            # --- stage out + pair AllGather of the vt half ---
            for j in range(HT):
                nc.sync.dma_start(stage_d[j, :, :], vt_own[:, j, :])
            nc.gpsimd.collective_compute(
                "AllGather",
                mybir.AluOpType.bypass,
                replica_groups=GROUPS,
                ins=[stage_d[:, :, :]],
                outs=[gath_d[:, :, :, :]],
            )

            # --- qM projection (hides the collective) ---
            w_m = wpool.tile([P, HT, H], BF16, tag="w", name="w_m")
            for j in range(HT):
                nc.sync.dma_start(w_m[:, j, :], M_v[:, j, :])
            for i in range(HT):
                pss = [
                    p1ps.tile([P, 512], F32, tag="ps", name=f"psq_{i}_{s}")
                    for s in range(2)
                ]
                for j in range(HT):
                    for s in range(2):
                        nc.tensor.matmul(
                            pss[s][:, :],
                            w_m[:, j, ts(i, P)],
                            xTq_sb[:, j, ds(s * 512, 512)],
                            start=(j == 0),
                            stop=(j == HT - 1),
                        )
                for s in range(2):
                    nc.scalar.activation(
                        qMT_sb[:, i, ds(s * 512, 512)], pss[s][:, :], AF.Identity,
                        bias=u_sb[:, i : i + 1], scale=1.0,
                    )

            # --- reload gathered full vt ---
            for h in range(2):
                for j in range(HT):
                    nc.sync.dma_start(vt_sb[:, h * HT + j, :], gath_d[h, j, :, :])
        w_es.close()
        x_es.close()

        # ---------------- Phase 3: scoresT -> exp -> st (centered fp8) ----------------
        st_es = ExitStack()
        stp = st_es.enter_context(tc.tile_pool(name="stp", bufs=1, side="right"))
        st_sb = stp.tile([P, ST, SQ], AV8, name="st_sb")
        with tc.tile_pool(name="p3ps", bufs=4, space="PSUM") as p3ps, \
             tc.tile_pool(name="p3st", bufs=4) as p3st:
            for t in range(ST):
                pss = [
                    p3ps.tile([P, 512], F32, tag="ps3", name=f"ps3_{t}_{qc}")
                    for qc in range(2)
                ]
                if P3_FP8:
                    for jp in range(HT // 2):
                        for qc in range(2):
                            nc.tensor.matmul(
                                pss[qc][:, :],
                                xTf_sb[:, ds(2 * jp, 2), ts(t, P)],
                                qMT_sb[:, ds(2 * jp, 2), ds(qc * 512, 512)],
                                start=(jp == 0),
                                stop=(jp == HT // 2 - 1),
                                perf_mode=DR,
                            )
                else:
                    for j in range(HT):
                        for qc in range(2):
                            nc.tensor.matmul(
                                pss[qc][:, :],
                                xTf_sb[:, j, ts(t, P)],
                                qMT_sb[:, j, ds(qc * 512, 512)],
                                start=(j == 0),
                                stop=(j == HT - 1),
                            )
                for qc in range(2):
                    if P4_FP8:
                        st32 = p3st.tile([P, 512], F32, tag="st32", name=f"st32_{t}_{qc}")
                        nc.scalar.activation(
                            st32[:, :], pss[qc][:, :], AF.Exp, bias=0.0, scale=SCALE,
                        )
                        nc.vector.tensor_scalar_add(
                            st_sb[:, t, ds(qc * 512, 512)], st32[:, :], -1.0,
                        )
                    else:
                        nc.scalar.activation(
                            st_sb[:, t, ds(qc * 512, 512)], pss[qc][:, :], AF.Exp,
                            bias=0.0, scale=SCALE,
                        )
        kq_es.close()  # free xTf/qMT

        # ---------------- Phase 4: out[q,h] = st^T @ vt (+sums column) ----------------
        with tc.tile_pool(name="p4w", bufs=1) as p4w, \
             tc.tile_pool(name="p4f", bufs=3) as p4f, \
             tc.tile_pool(name="p4ps", bufs=2, space="PSUM") as p4ps, \
             tc.tile_pool(name="p4sum", bufs=1, space="PSUM") as p4sum, \
             tc.tile_pool(name="p4cs", bufs=1, space="PSUM") as p4cs:
            sums_ps = p4sum.tile([P, HT], F32, name="sums_ps")
            invs_sb = p4w.tile([P, HT], F32, name="invs_sb")
            if P4_FP8:
                # colsum(vt8) broadcast over partitions, via all-ones stationary
                cs_ps = p4cs.tile([P, H], F32, name="cs_ps")
                for tp in range(ST // 2):
                    for hc in range(2):
                        nc.tensor.matmul(
                            cs_ps[:, ds(hc * 512, 512)],
                            onesW[:, :, :],
                            vt_sb[:, ds(2 * tp, 2), ds(hc * 512, 512)],
                            start=(tp == 0),
                            stop=(tp == ST // 2 - 1),
                            perf_mode=DR,
                        )
                colsum_sb = p4w.tile([P, H], F32, name="colsum_sb")
                nc.scalar.activation(colsum_sb[:, :], cs_ps[:, :], AF.Copy)

            for qt in range(SQ // P):
                ps = p4ps.tile([P, H], F32, tag="ps4", name="ps4")
                if P4_FP8:
                    for tp in range(ST // 2):
                        st_w = st_sb[:, ds(2 * tp, 2), ts(qt, P)]
                        for hc in range(2):
                            nc.tensor.matmul(
                                ps[:, ds(hc * 512, 512)],
                                st_w,
                                vt_sb[:, ds(2 * tp, 2), ds(hc * 512, 512)],
                                start=(tp == 0),
                                stop=(tp == ST // 2 - 1),
                                perf_mode=DR,
                            )
                        nc.tensor.matmul(
                            sums_ps[:, qt : qt + 1],
                            st_w,
                            ones3[:, :, 0:1],
                            start=(tp == 0),
                            stop=(tp == ST // 2 - 1),
                            perf_mode=DR,
                        )
                else:
                    for t in range(ST):
                        st_w = st_sb[:, t, ts(qt, P)]
                        for hc in range(2):
                            nc.tensor.matmul(
                                ps[:, ds(hc * 512, 512)],
                                st_w,
                                vt_sb[:, t, ds(hc * 512, 512)],
                                start=(t == 0),
                                stop=(t == ST - 1),
                            )
                        nc.tensor.matmul(
                            sums_ps[:, qt : qt + 1],
                            st_w,
                            ones3[:, 0, 0:1],
                            start=(t == 0),
                            stop=(t == ST - 1),
                        )
                # epilogue: invs, (+colsum), scale, +beff, store
                if P4_FP8:
                    nc.vector.tensor_scalar_add(
                        invs_sb[:, qt : qt + 1], sums_ps[:, qt : qt + 1], float(S)
                    )
                    nc.vector.reciprocal(
                        invs_sb[:, qt : qt + 1], invs_sb[:, qt : qt + 1]
                    )
                else:
                    nc.vector.reciprocal(
                        invs_sb[:, qt : qt + 1], sums_ps[:, qt : qt + 1]
                    )
                fin32 = p4f.tile([P, H], F32, tag="fin32", name="fin32")
                if P4_FP8:
                    nc.vector.tensor_add(fin32[:, :], ps[:, :], colsum_sb[:, :])
                    src = fin32
                else:
                    src = ps
                fin = p4f.tile([P, H], F32, tag="fin", name="fin")
                nc.scalar.activation(
                    fin[:, :], src[:, :], AF.Copy, bias=0.0,
                    scale=invs_sb[:, qt : qt + 1],
                )
                nc.vector.tensor_add(fin[:, :], fin[:, :], beff_sb[:, :])
                nc.sync.dma_start(out_d.ap()[ts(qt, P), :], fin[:, :])
        st_es.close()
        v_es.close()

    _split_excess_waits(nc)
    return nc


_NC_CACHE = None


def _get_nc():
    global _NC_CACHE
    if _NC_CACHE is None:
        _NC_CACHE = _build_nc()
    return _NC_CACHE


def _make_in_maps(x, w_attn, b_attn, w_proj, b_proj):
    B = x.shape[0]
    KNP = ml_dtypes.float8_e4m3 if P3_FP8 else ml_dtypes.bfloat16
    wq = w_attn[:, :H].astype(np.float64)
    wk = w_attn[:, H : 2 * H].astype(np.float64)
    wv = w_attn[:, 2 * H :].astype(np.float64)
    wp = w_proj.astype(np.float64)
    M16 = np.ascontiguousarray((wq @ wk.T).astype(np.float32)).astype(ml_dtypes.bfloat16)
    wvt16 = np.ascontiguousarray((wv @ wp).astype(np.float32)).astype(ml_dtypes.bfloat16)
    u = (wk @ b_attn[:H].astype(np.float64)).astype(np.float32)
    u_cols = np.ascontiguousarray(u.reshape(HT, P).T)
    beff = (
        b_attn[2 * H :].astype(np.float64) @ wp + b_proj.astype(np.float64)
    ).astype(np.float32)
    beff_b = np.ascontiguousarray(np.broadcast_to(beff, (P, H)))
    in_maps = []
    xTs = [np.ascontiguousarray(x[b].T.astype(np.float32)) for b in range(B)]
    xT8s = [np.clip(t, -240.0, 240.0).astype(KNP) for t in xTs]
    xTq16 = [t.astype(ml_dtypes.bfloat16) for t in xTs]
    for c in range(2 * B):
        b, h = c // 2, c % 2
        in_maps.append(
            {
                "xTq": np.ascontiguousarray(xTq16[b][:, h * SQ : (h + 1) * SQ]),
                "xTf": xT8s[b],
                "Mw": M16,
                "wvt": wvt16,
                "u_cols": u_cols,
                "beff_bcast": beff_b,
            }
        )
    return in_maps


def kernel(x, w_attn, b_attn, w_proj, b_proj, _trace=False, _trace_kwargs=None):
    x = np.asarray(x, dtype=np.float32)
    B, S_, H_ = x.shape
    nc = _get_nc()
    in_maps = _make_in_maps(
        x, np.asarray(w_attn), np.asarray(b_attn),
        np.asarray(w_proj), np.asarray(b_proj),
    )
    kw = {}
    if _trace:
        kw["trace"] = True
        if _trace_kwargs:
            kw.update(_trace_kwargs)
    res = run_bass_kernel_spmd(nc, in_maps, core_ids=list(range(2 * B)), **kw)
    out = np.empty((B, S_, H_), np.float32)
    for c in range(2 * B):
        b, h = c // 2, c % 2
        out[b, h * SQ : (h + 1) * SQ, :] = res.results[c]["out"]
    if _trace:
        kernel._last_results = res
    return out


if __name__ == "__main__":
    rng = np.random.default_rng(0)
    x = rng.standard_normal((4, S, H), dtype=np.float32)
    w_attn = rng.standard_normal((H, 3 * H), dtype=np.float32) * 0.02
    b_attn = rng.standard_normal((3 * H,), dtype=np.float32) * 0.02
    w_proj = rng.standard_normal((H, H), dtype=np.float32) * 0.02
    b_proj = rng.standard_normal((H, ), dtype=np.float32) * 0.02
    out = kernel(x=x, w_attn=w_attn, b_attn=b_attn, w_proj=w_proj, b_proj=b_proj)
    print("out", out.shape, out.dtype, float(np.abs(out).max()))



# revision 2
# speedup vs baseline: 1.0550x; 1.0550x over previous
# Trainium2 Bass kernel for a single-head attention block:
#   qkv = x @ w_attn + b_attn ; q,k,v = split(qkv)
#   out = softmax(q @ k.T / sqrt(H)) @ v @ w_proj + b_proj
# Shapes: x [4, 2048, 1024], w_attn [1024, 3072], w_proj [1024, 1024], f32.
#
# Weight folding (host, input-independent):
#   M    = w_q @ w_k^T          -> scores = x @ M @ x^T (+ column bias)
#   u    = w_k @ b_q            -> the only softmax-surviving bias term
#   w_vt = w_v @ w_proj         -> out = attn @ (x @ w_vt) + beff
#   beff = b_v @ w_proj + b_proj  (folded INTO vt rows: softmax weights
#                                  sum to 1, so attn @ (vt + 1 beff^T)
#                                  = attn @ vt + beff exactly)
# (row-dependent bias terms cancel inside softmax). This removes the K
# projection and the output projection: 12.9 GFLOP/core instead of 19.45.
#
# Sharding: 8 cores = 4 batches x 2 sequence-halves. Each core projects
# qM for its query half and vt for its own half; core pairs exchange vt
# halves with an intra-pair AllGather (hidden under the qM projection).
#
# Precision: projections run in bf16; the two attention matmuls run in
# fp8e4m3 with DoubleRow (2 k-tiles per pass). st is stored centered
# (st-1) in fp8 to halve its quantization error; the exact rank-1
# correction ones x colsum(vt8) is preloaded into PSUM (scalar copy +
# start=False accumulation) so the P4 epilogue is a single scaled copy.
#
# Schedule: Phase-1 runs j-outer (contraction-outer) over 8 PSUM banks
# so the PE consumes each w/x DMA chunk as it lands; P4 preloads are
# prefetched one q-tile ahead to keep the scalar queue off the critical
# path; epilogues are half-tile (512 cols) so the drain after the last
# matmul is ~1.5us.
import numpy as np
import ml_dtypes

import concourse.bass as bass
import concourse.mybir as mybir
import concourse.tile as tile
from concourse.bass import ts, ds
from concourse.bass_utils import run_bass_kernel_spmd

P = 128
H = 1024
S = 2048
SQ = 1024  # sequence rows per core (half)
HT = H // P  # 8 h-tiles
ST = S // P  # 16 s-tiles
F32 = mybir.dt.float32
BF16 = mybir.dt.bfloat16
FP8 = mybir.dt.float8e4
AF = mybir.ActivationFunctionType
DR = mybir.MatmulPerfMode.DoubleRow
SCALE = 1.0 / 32.0  # 1/sqrt(H)
GROUPS = [[0, 1], [2, 3], [4, 5], [6, 7]]


def _split_excess_waits(nc, limit=1):
    """walrus codegen allows very few sync-wait commands per instruction
    (1 is safe for generic ops, 0 for collectives). Move overflow waits
    onto injected same-engine NoOps just before the offending instruction
    (engines are in-order, so this is equivalent)."""
    n_split = 0
    for f in nc.m.functions:
        for blk in f.blocks:
            il = blk.instructions
            def _limit(inst):
                return 0 if type(inst).__name__ == "InstCollectiveCompute" else limit
            if not any(
                i.sync_info and i.sync_info.on_wait
                and len(i.sync_info.on_wait) > _limit(i)
                for i in il
            ):
                continue
            newl = []
            for inst in il:
                si = inst.sync_info
                lim = _limit(inst)
                if si is not None and si.on_wait and len(si.on_wait) > lim:
                    waits = list(si.on_wait)
                    while len(waits) > lim:
                        take = max(1, limit)
                        chunk, waits = waits[:take], waits[take:]
                        nop = mybir.InstNoOp(
                            name=f"{inst.name}-wsplit{n_split}", ins=[], outs=[]
                        )
                        nop.engine = inst.engine
                        nop.sync_info = mybir.SyncInfo(on_wait=chunk, on_update=[])
                        newl.append(nop)
                        n_split += 1
                    inst.sync_info = mybir.SyncInfo(
                        on_wait=waits, on_update=list(si.on_update)
                    )
                newl.append(inst)
            il[:] = newl
            assert len(blk.instructions) == len(newl)
    return n_split


def _build_nc():
    nc = bass.Bass("TRN2", target_bir_lowering=False, debug=False, num_devices=8)

    xTq_d = nc.dram_tensor("xTq", [H, SQ], BF16, kind="ExternalInput")
    xTf_d = nc.dram_tensor("xTf", [H, S], FP8, kind="ExternalInput")
    M_d = nc.dram_tensor("Mw", [H, H], BF16, kind="ExternalInput")
    wvt_d = nc.dram_tensor("wvt", [H, H], BF16, kind="ExternalInput")
    u_d = nc.dram_tensor("u_cols", [P, HT], F32, kind="ExternalInput")
    beff_d = nc.dram_tensor("beff_bcast", [P, H], F32, kind="ExternalInput")
    out_d = nc.dram_tensor("out", [SQ, H], F32, kind="ExternalOutput")

    xTq_v = xTq_d.ap().rearrange("(j p) s -> p j s", p=P)
    xTf_v = xTf_d.ap().rearrange("(j p) s -> p j s", p=P)
    M_v = M_d.ap().rearrange("(j p) c -> p j c", p=P)
    wvt_v = wvt_d.ap().rearrange("(j p) c -> p j c", p=P)

    from contextlib import ExitStack

    with tile.TileContext(nc) as tc, ExitStack() as top:
        misc = top.enter_context(tc.tile_pool(name="misc", bufs=1))
        dpool = top.enter_context(tc.tile_pool(name="dram", bufs=1, space="DRAM"))

        # staging for the pair AllGather of the vt half (by s-tile)
        stage_d = dpool.tile([HT, P, H], FP8, name="stage_d")
        gath_d = dpool.tile([2, HT, P, H], FP8, name="gath_d")

        u_sb = misc.tile([P, HT], F32, name="u_sb")
        nc.sync.dma_start(u_sb[:, :], u_d.ap())
        beff_sb = misc.tile([P, H], F32, name="beff_sb")
        nc.sync.dma_start(beff_sb[:, :], beff_d.ap())

        # fp8 ones for the sums column (middle-dim step 16 to satisfy DR APs)
        ones3 = misc.tile([P, 2, 16], FP8, name="ones3")
        nc.vector.memset(ones3[:, :, :], 1.0)
        # all-ones stationary for the colsum broadcast matmul
        onesW = misc.tile([P, 2, P], FP8, name="onesW")
        nc.vector.memset(onesW[:, :, :], 1.0)

        # persistent operand buffers
        kq_es = ExitStack()
        kq = kq_es.enter_context(tc.tile_pool(name="kq", bufs=1))
        xTf_sb = kq.tile([P, HT, S], FP8, name="xTf_sb")
        qMT_sb = kq.tile([P, HT, SQ], FP8, name="qMT_sb")
        v_es = ExitStack()
        vp = v_es.enter_context(tc.tile_pool(name="vp", bufs=1, side="right"))
        vt_sb = vp.tile([P, ST, H], FP8, name="vt_sb")

        x_es = ExitStack()
        xTp = x_es.enter_context(tc.tile_pool(name="xTp", bufs=1))
        w_es = ExitStack()
        wpool = w_es.enter_context(tc.tile_pool(name="wpool", bufs=2))

        xTq_sb = xTp.tile([P, HT, SQ], BF16, name="xTq_sb")
        w_v = wpool.tile([P, HT, H], BF16, tag="w", name="w_v")
        # critical chunks first: P1 consumes (w_v[j], xTq[j]) j-outer
        for j in range(HT):
            nc.sync.dma_start(w_v[:, j, :], wvt_v[:, j, :])
            nc.sync.dma_start(xTq_sb[:, j, :], xTq_v[:, j, :])

        warm_sb = misc.tile([P, 512], BF16, name="warm_sb")
        nc.vector.memset(warm_sb[:, :], 1.0)

        vt_own = xTp.tile([P, HT, H], FP8, name="vt_own")

        # ---------------- PSUM pool for warm-up / P1 / qM / P3 ----------------
        p123_es = ExitStack()
        mmps = p123_es.enter_context(tc.tile_pool(name="mmps", bufs=8, space="PSUM"))

        # PE warm-up on const data while the first DMAs land: keeps the
        # HAM activity window full so real matmuls start at 2.4 GHz.
        for wi in range(2):
            wps = mmps.tile([P, 512], F32, tag="mm", name=f"warm_ps{wi}")
            for r in range(8):
                nc.tensor.matmul(
                    wps[:, :],
                    warm_sb[:, 0:P],
                    warm_sb[:, :],
                    start=(r == 0),
                    stop=(r == 7),
                )

        # --- Phase 1: vt own half, j-outer over 8 PSUM banks ---
        # lhsT = xTq s-tile (stationary), rhs = w_vt half-row; the PE
        # consumes each (w_v[j], xTq[j]) DMA chunk as it lands.
        for hc in range(2):
            psv = [
                mmps.tile([P, 512], F32, tag="mm", name=f"psv_{hc}_{si}")
                for si in range(HT)
            ]
            for j in range(HT):
                for si in range(HT):
                    nc.tensor.matmul(
                        psv[si][:, :],
                        xTq_sb[:, j, ts(si, P)],
                        w_v[:, j, ds(hc * 512, 512)],
                        start=(j == 0),
                        stop=(j == HT - 1),
                    )
            # epilogue: add beff (folded bias) and store fp8
            for si in range(HT):
                nc.vector.tensor_add(
                    vt_own[:, si, ds(hc * 512, 512)],
                    psv[si][:, :],
                    beff_sb[:, ds(hc * 512, 512)],
                )

        # --- stage out + pair AllGather of the vt half ---
        for j in range(HT):
            nc.sync.dma_start(stage_d[j, :, :], vt_own[:, j, :])
        nc.gpsimd.collective_compute(
            "AllGather",
            mybir.AluOpType.bypass,
            replica_groups=GROUPS,
            ins=[stage_d[:, :, :]],
            outs=[gath_d[:, :, :, :]],
        )

        # --- qM projection (hides the collective) ---
        w_m = wpool.tile([P, HT, H], BF16, tag="w", name="w_m")
        for j in range(HT):
            nc.sync.dma_start(w_m[:, j, :], M_v[:, j, :])
        for j in range(HT):
            nc.sync.dma_start(xTf_sb[:, j, :], xTf_v[:, j, :])
        for i in range(HT):
            pss = [
                mmps.tile([P, 512], F32, tag="mm", name=f"psq_{i}_{s}")
                for s in range(2)
            ]
            for j in range(HT):
                for s in range(2):
                    nc.tensor.matmul(
                        pss[s][:, :],
                        w_m[:, j, ts(i, P)],
                        xTq_sb[:, j, ds(s * 512, 512)],
                        start=(j == 0),
                        stop=(j == HT - 1),
                    )
            for s in range(2):
                nc.scalar.activation(
                    qMT_sb[:, i, ds(s * 512, 512)], pss[s][:, :], AF.Identity,
                    bias=u_sb[:, i : i + 1], scale=1.0,
                )

        # --- reload gathered full vt ---
        for h in range(2):
            for j in range(HT):
                nc.sync.dma_start(vt_sb[:, h * HT + j, :], gath_d[h, j, :, :])
        w_es.close()
        x_es.close()

        # ---------------- Phase 3: scoresT -> exp -> st (centered fp8) ----------------
        st_es = ExitStack()
        stp = st_es.enter_context(tc.tile_pool(name="stp", bufs=1, side="right"))
        st_sb = stp.tile([P, ST, SQ], FP8, name="st_sb")
        with tc.tile_pool(name="p3st", bufs=4) as p3st:
            for t in range(ST):
                pss = [
                    mmps.tile([P, 512], F32, tag="mm", name=f"ps3_{t}_{qc}")
                    for qc in range(2)
                ]
                for jp in range(HT // 2):
                    for qc in range(2):
                        nc.tensor.matmul(
                            pss[qc][:, :],
                            xTf_sb[:, ds(2 * jp, 2), ts(t, P)],
                            qMT_sb[:, ds(2 * jp, 2), ds(qc * 512, 512)],
                            start=(jp == 0),
                            stop=(jp == HT // 2 - 1),
                            perf_mode=DR,
                        )
                for qc in range(2):
                    st32 = p3st.tile([P, 512], F32, tag="st32", name=f"st32_{t}_{qc}")
                    nc.scalar.activation(
                        st32[:, :], pss[qc][:, :], AF.Exp, bias=0.0, scale=SCALE,
                    )
                    nc.vector.tensor_scalar_add(
                        st_sb[:, t, ds(qc * 512, 512)], st32[:, :], -1.0,
                    )
        p123_es.close()
        kq_es.close()  # free xTf/qMT

        # ---------------- Phase 4: out[q,h] = st^T @ vt (+sums column) ----------------
        with tc.tile_pool(name="p4w", bufs=1) as p4w, \
             tc.tile_pool(name="p4f", bufs=4) as p4f, \
             tc.tile_pool(name="p4ps", bufs=7, space="PSUM") as p4ps, \
             tc.tile_pool(name="p4sum", bufs=1, space="PSUM") as p4sum:
            sums_ps = p4sum.tile([P, HT], F32, name="sums_ps")
            invs_sb = p4w.tile([P, HT], F32, name="invs_sb")
            # colsum(vt8) broadcast over partitions, via all-ones stationary
            colsum_sb = p4w.tile([P, H], F32, name="colsum_sb")
            cs = [
                p4ps.tile([P, 512], F32, tag="mm4", name=f"cs_{hc}")
                for hc in range(2)
            ]
            for tp in range(ST // 2):
                for hc in range(2):
                    nc.tensor.matmul(
                        cs[hc][:, :],
                        onesW[:, :, :],
                        vt_sb[:, ds(2 * tp, 2), ds(hc * 512, 512)],
                        start=(tp == 0),
                        stop=(tp == ST // 2 - 1),
                        perf_mode=DR,
                    )
            for hc in range(2):
                nc.scalar.activation(
                    colsum_sb[:, ds(hc * 512, 512)], cs[hc][:, :], AF.Copy
                )

            # PSUM tiles are preloaded with colsum (scalar copy), matmuls
            # accumulate on top with start=False; preloads are prefetched
            # one q-tile ahead so the scalar queue stays off the PE path.
            def new_preloaded(qt):
                pair = []
                for hc in range(2):
                    t4 = p4ps.tile([P, 512], F32, tag="mm4", name=f"ps4_{qt}_{hc}")
                    nc.scalar.activation(
                        t4[:, :], colsum_sb[:, ds(hc * 512, 512)], AF.Copy
                    )
                    pair.append(t4)
                return pair

            ps4 = new_preloaded(0)
            for qt in range(SQ // P):
                for hc in range(2):
                    for tp in range(ST // 2):
                        st_w = st_sb[:, ds(2 * tp, 2), ts(qt, P)]
                        nc.tensor.matmul(
                            ps4[hc][:, ds(0, 512)],
                            st_w,
                            vt_sb[:, ds(2 * tp, 2), ds(hc * 512, 512)],
                            start=False,
                            stop=(tp == ST // 2 - 1),
                            perf_mode=DR,
                            skip_group_check=True,
                        )
                        if hc == 0:
                            nc.tensor.matmul(
                                sums_ps[:, qt : qt + 1],
                                st_w,
                                ones3[:, :, 0:1],
                                start=(tp == 0),
                                stop=(tp == ST // 2 - 1),
                                perf_mode=DR,
                            )
                    if hc == 0:
                        nc.vector.tensor_scalar_add(
                            invs_sb[:, qt : qt + 1], sums_ps[:, qt : qt + 1],
                            float(S),
                        )
                        nc.vector.reciprocal(
                            invs_sb[:, qt : qt + 1], invs_sb[:, qt : qt + 1]
                        )
                cur, ps4 = ps4, (new_preloaded(qt + 1) if qt + 1 < SQ // P else None)
                for hc in range(2):
                    fin = p4f.tile([P, 512], F32, tag="fin", name=f"fin_{qt}_{hc}")
                    nc.scalar.activation(
                        fin[:, :], cur[hc][:, :], AF.Copy, bias=0.0,
                        scale=invs_sb[:, qt : qt + 1],
                    )
                    nc.sync.dma_start(
                        out_d.ap()[ts(qt, P), ds(hc * 512, 512)], fin[:, :]
                    )
        st_es.close()
        v_es.close()

    _split_excess_waits(nc)
    return nc


_NC_CACHE = None


def _get_nc():
    global _NC_CACHE
    if _NC_CACHE is None:
        _NC_CACHE = _build_nc()
    return _NC_CACHE


def _make_in_maps(x, w_attn, b_attn, w_proj, b_proj):
    B = x.shape[0]
    wq = w_attn[:, :H].astype(np.float64)
    wk = w_attn[:, H : 2 * H].astype(np.float64)
    wv = w_attn[:, 2 * H :].astype(np.float64)
    wp = w_proj.astype(np.float64)
    M16 = np.ascontiguousarray((wq @ wk.T).astype(np.float32)).astype(ml_dtypes.bfloat16)
    wvt16 = np.ascontiguousarray((wv @ wp).astype(np.float32)).astype(ml_dtypes.bfloat16)
    u = (wk @ b_attn[:H].astype(np.float64)).astype(np.float32)
    u_cols = np.ascontiguousarray(u.reshape(HT, P).T)
    beff = (
        b_attn[2 * H :].astype(np.float64) @ wp + b_proj.astype(np.float64)
    ).astype(np.float32)
    beff_b = np.ascontiguousarray(np.broadcast_to(beff, (P, H)))
    in_maps = []
    xTs = [np.ascontiguousarray(x[b].T.astype(np.float32)) for b in range(B)]
    xT8s = [np.clip(t, -240.0, 240.0).astype(ml_dtypes.float8_e4m3) for t in xTs]
    xTq16 = [t.astype(ml_dtypes.bfloat16) for t in xTs]
    for c in range(2 * B):
        b, h = c // 2, c % 2
        in_maps.append(
            {
                "xTq": np.ascontiguousarray(xTq16[b][:, h * SQ : (h + 1) * SQ]),
                "xTf": xT8s[b],
                "Mw": M16,
                "wvt": wvt16,
                "u_cols": u_cols,
                "beff_bcast": beff_b,
            }
        )
    return in_maps


def kernel(x, w_attn, b_attn, w_proj, b_proj, _trace=False, _trace_kwargs=None):
    x = np.asarray(x, dtype=np.float32)
    B, S_, H_ = x.shape
    nc = _get_nc()
    in_maps = _make_in_maps(
        x, np.asarray(w_attn), np.asarray(b_attn),
        np.asarray(w_proj), np.asarray(b_proj),
    )
    kw = {}
    if _trace:
        kw["trace"] = True
        if _trace_kwargs:
            kw.update(_trace_kwargs)
    res = run_bass_kernel_spmd(nc, in_maps, core_ids=list(range(2 * B)), **kw)
    out = np.empty((B, S_, H_), np.float32)
    for c in range(2 * B):
        b, h = c // 2, c % 2
        out[b, h * SQ : (h + 1) * SQ, :] = res.results[c]["out"]
    if _trace:
        kernel._last_results = res
    return out


if __name__ == "__main__":
    rng = np.random.default_rng(0)
    x = rng.standard_normal((4, S, H), dtype=np.float32)
    w_attn = rng.standard_normal((H, 3 * H), dtype=np.float32) * 0.02
    b_attn = rng.standard_normal((3 * H,), dtype=np.float32) * 0.02
    w_proj = rng.standard_normal((H, H), dtype=np.float32) * 0.02
    b_proj = rng.standard_normal((H, ), dtype=np.float32) * 0.02
    out = kernel(x=x, w_attn=w_attn, b_attn=b_attn, w_proj=w_proj, b_proj=b_proj)
    print("out", out.shape, out.dtype, float(np.abs(out).max()))
